# revision 1
# baseline (speedup 1.0000x reference)
"""v2: host pre-transposes x per 128-row batch tile -> xt [bt, m, i, b128].

Device kernel per core: pure fp32 matmuls (no PE transposes, no DVE
transpose-copies):
  per bt (8): one rearranged 2MiB DMA load xt[bt] -> SBUF [128i, (m k), 128b]
  per m: 2 accumulating matmuls (lhsT = xT slice [128i,128b] stationary,
         rhs = w[m] k-half [128i, 256o] moving) -> PSUM [128b, 256o]
  DVE copy -> staging [128b, 16m, 256o], one 2MiB store per bt.
"""

import numpy as np
from contextlib import ExitStack

import concourse.bass as bass
import concourse.tile as tile
import concourse.mybir as mybir
from concourse import bacc
from concourse.bass import ts
from concourse.bass_utils import run_bass_kernel_spmd

BATCH, M, D_IN, D_OUT = 8192, 16, 256, 256
N_CORES = 8
P = 128
F32 = mybir.dt.float32


def build_program(b_per_core: int, repeat: int = 1) -> bass.Bass:
    """repeat>1 re-runs the whole body (idempotent) — used only to measure
    true device time as the wall-clock slope over repeats."""
    nc = bacc.Bacc("TRN2", target_bir_lowering=False, debug=False)

    n_btiles = b_per_core // P
    KT = D_IN // P  # 2

    xt_ap = nc.dram_tensor(
        "xt", [n_btiles, M, D_IN, P], F32, kind="ExternalInput"
    ).ap()
    w_ap = nc.dram_tensor("w", [M, D_IN, D_OUT], F32, kind="ExternalInput").ap()
    o_ap = nc.dram_tensor("out", [b_per_core, M, D_OUT], F32, kind="ExternalOutput").ap()

    with tile.TileContext(nc) as tc, ExitStack() as ctx:
        w_pool = ctx.enter_context(tc.tile_pool(name="w", bufs=1))
        x_pool = ctx.enter_context(tc.tile_pool(name="x", bufs=3))
        o_pool = ctx.enter_context(tc.tile_pool(name="o", bufs=3))
        pso_pool = ctx.enter_context(tc.tile_pool(name="pso", bufs=8, space="PSUM"))

        # Resident weights: [128i, (m k), 256o] — single DMA (1-wait limit).
        w_sb = w_pool.tile([P, M * KT, D_OUT], F32)
        nc.sync.dma_start(
            out=w_sb[:], in_=w_ap.rearrange("m (k p) o -> p (m k) o", p=P)
        )

        for bt_r in range(n_btiles * repeat):
            bt = bt_r % n_btiles
            xts = x_pool.tile([P, M * KT, P], F32)
            nc.sync.dma_start(
                out=xts[:],
                in_=xt_ap[bt].rearrange("m (k p) b -> p (m k) b", p=P),
            )
            ot = o_pool.tile([P, M, D_OUT], F32)

            for m in range(M):
                ps_o = pso_pool.tile([P, D_OUT], F32)
                for k in range(KT):
                    nc.tensor.matmul(
                        ps_o[:],
                        lhsT=xts[:, m * KT + k, :],
                        rhs=w_sb[:, m * KT + k, :],
                        start=(k == 0),
                        stop=(k == KT - 1),
                    )
                nc.vector.tensor_copy(out=ot[:, m, :], in_=ps_o[:])

            nc.sync.dma_start(out=o_ap[ts(bt, P)], in_=ot[:])

    nc.compile()
    return nc


def _host_transpose(x_shard: np.ndarray) -> np.ndarray:
    b = x_shard.shape[0]
    # [b, m, i] -> [bt, m, i, 128b]
    return np.ascontiguousarray(
        x_shard.reshape(b // P, P, M, D_IN).transpose(0, 2, 3, 1)
    )


def _run(x: np.ndarray, weights: np.ndarray, trace: bool = False):
    b_per_core = x.shape[0] // N_CORES
    nc = build_program(b_per_core)
    shards = np.split(x, N_CORES, axis=0)
    w = np.ascontiguousarray(weights)
    in_maps = [{"xt": _host_transpose(s), "w": w} for s in shards]
    res = run_bass_kernel_spmd(nc, in_maps, list(range(N_CORES)), trace=trace)
    out = np.concatenate([r["out"] for r in res.results], axis=0)
    return out, res


def kernel(x: np.ndarray, weights: np.ndarray) -> np.ndarray:
    out, _ = _run(np.asarray(x), np.asarray(weights), trace=False)
    return out

